# revision 1
# baseline (speedup 1.0000x reference)
"""Dilated (3x3, dilation=2) local-window attention for Trainium2.

Full inputs: x (32, 3136, 96) f32, W_qkv (288, 96) f32.
Sharding: data-parallel over batch, 4 images per core on 8 cores.

Key structure: with dilation 2, the 56x56 image splits into 4 independent
28x28 parity sub-lattices, each with an ordinary 3x3 dilation-1 window
(zero-padded by 1).  Host repacks x into padded parity layout
xt[97, par, 32, 30] (channel 96 = ones row driving the softmax
denominator; rows 0/29..31 and cols 0/29 are zero pads).

Per parity sub-image (28 sub-rows x 28 cols):
  - g = wqkL^T x (fused q^T k scores: S = x_pos^T g_tok), 2 chunk matmuls.
  - v windows: 8 matmuls [120 pos = 4 padded rows x 30, 97] (col 96 = den
    ones channel), drained once to bf16 pv (non-overlapping row groups).
  - S: 14 block matmuls [120 pos, 56 tok] (block = 2 sub-rows; window =
    4 padded rows x 30), packed 7 blocks/PSUM bank.
  - exp on ACT over [120, 2x392] (scale folded), constant band mask on
    DVE (one [120, 56] tile broadcast over the 14 blocks).
  - AV: out[97, 56] per block; even blocks hit one pv row group, odd
    blocks accumulate two 60-partition halves.
  - drain to bf16, DMA out; final num/den division on host.
"""

import numpy as np
import ml_dtypes

import concourse.bass as bass
import concourse.bacc as bacc
import concourse.tile as tile
from concourse import mybir
from concourse.bass_utils import run_bass_kernel_spmd

BF16 = mybir.dt.bfloat16
F32 = mybir.dt.float32

B = 32
NCORES = 8
BPC = B // NCORES   # images per core
H = 56
C = 96
N = H * H
SCALE = C ** -0.5
P = 4               # parity classes
R = 28              # sub-rows / cols per parity image
PR = 32             # padded width (window = 4 x 32 = 128 partitions)
RP = 32             # padded rows (row -1, 28 real, pad, 2 dummy)
NB = 14             # 2-sub-row blocks per parity
TOK = R * R         # 784 tokens per parity

_NC_CACHE = {}

# greedy per-engine load balancing for PSUM->SBUF drains
_ACT, _DVE, _POOL = 0, 1, 2


class _Balancer:
    def __init__(self):
        self.load = [0.0, 0.0, 0.0]

    def charge(self, eng, ns):
        self.load[eng] += ns

    def pick(self, free):
        # PSUM->SBUF drains: GPSIMD cannot touch PSUM, so ACT/DVE only
        costs = (free * 0.8333 + 185.0,   # ACT
                 free * 1.0417 + 125.0)   # DVE
        best = min(range(2), key=lambda e: self.load[e] + costs[e])
        self.load[best] += costs[best]
        return best


def _copy(nc, eng, dst, src):
    if eng == _ACT:
        nc.scalar.copy(dst, src)
    elif eng == _DVE:
        nc.vector.tensor_copy(dst, src)
    else:
        nc.gpsimd.tensor_copy(dst, src)


def build_nc():
    nc = bacc.Bacc("TRN2", target_bir_lowering=False)
    xt_d = nc.dram_tensor("xt", [BPC, C + 1, P, RP, PR], BF16, kind="ExternalInput")
    wt_d = nc.dram_tensor("wt", [C + 1, 2 * C + 1], BF16, kind="ExternalInput")
    mk_d = nc.dram_tensor("mask", [128, NB, 56], BF16, kind="ExternalInput")
    o_d = nc.dram_tensor("o", [BPC, P, C + 1, TOK], BF16, kind="ExternalOutput")

    with tile.TileContext(nc) as tc:
        _body(tc, xt_d, wt_d, mk_d, o_d)
    nc.compile()
    return nc


def _body(tc, xt_d, wt_d, mk_d, o_d):
    nc = tc.nc
    bal = _Balancer()
    # fixed per-parity work charged to its engine
    # exp: ACT, mask: DVE, out drains split DVE/POOL
    with (
        tc.tile_pool(name="const", bufs=1) as const,
        tc.tile_pool(name="xpool", bufs=2) as xpool,
        tc.tile_pool(name="gpool", bufs=2) as gpool,
        tc.tile_pool(name="pvpool", bufs=2) as pvpool,
        tc.tile_pool(name="epool", bufs=2) as epool,
        tc.tile_pool(name="opool", bufs=2) as opool,
        tc.tile_pool(name="psq", bufs=1, space="PSUM") as psq,
        tc.tile_pool(name="psv", bufs=1, space="PSUM") as psv,
        tc.tile_pool(name="pss", bufs=1, space="PSUM") as pss,
        tc.tile_pool(name="pso", bufs=1, space="PSUM") as pso,
    ):
        w_sb = const.tile([C + 1, 2 * C + 1], BF16)
        nc.sync.dma_start(w_sb[:], wt_d[:])
        wqkL = w_sb[0:C, 0:C]              # lhsT for g = (wq^T wk)^T x
        wv_ext = w_sb[:, C:2 * C + 1]      # [97, 97] v + den-ones channel
        m_sb = const.tile([128, NB, 56], BF16)
        nc.sync.dma_start(m_sb[:], mk_d[:])

        xtp = [None, None]

        def xwin(xt, nparts, par, row0, nrows):
            # contiguous [nparts, nrows*32] window of xtp [97, 4, 32, 32]
            return bass.AP(tensor=xt.tensor,
                           offset=xt.offset + par * (RP * PR) + row0 * PR,
                           ap=[[list(xt.ap[0])[0], nparts], [1, nrows * PR]])

        def load_image(b):
            xtp[b % 2] = xpool.tile([C + 1, P, RP, PR], BF16, tag="xtp", name="xtp")
            nc.sync.dma_start(
                xtp[b % 2].rearrange("p a b c -> p (a b c)"),
                xt_d[b].rearrange("p a b c -> p (a b c)"))

        def qk2_pass(b):
            """g = wqkL^T x over all 4 parities (interior tokens only)."""
            xt = xtp[b % 2]
            g = gpool.tile([C, P, R, R], BF16, tag="g")
            for par in range(P):
                for h in range(2):
                    q_ps = psq.tile([C, 512], F32, tag="q", name="q_ps")
                    nc.tensor.matmul(
                        q_ps[:, 0:392],
                        wqkL,
                        xt[0:C, par, 1 + 14 * h:15 + 14 * h, 1:29],
                        start=True, stop=True)
                    eng = bal.pick(392)
                    gs = g[:, par, 14 * h:14 * h + 14, :]
                    _copy(nc, eng,
                          bass.AP(tensor=gs.tensor, offset=gs.offset,
                                  ap=[list(gs.ap[0]), [1, 392]]),
                          q_ps[:, 0:392])
            return g

        def v_pass(b):
            """pv[120, 32, 97]: non-overlapping 4-padded-row v groups."""
            xt = xtp[b % 2]
            pv = pvpool.tile([128, P * 8, C + 1], BF16, tag="pv")
            for par in range(P):
                v_ps = psv.tile([128, 2, 512], F32, tag="v")
                for m in range(8):
                    nc.tensor.matmul(
                        v_ps[:, m // 4, 97 * (m % 4):97 * (m % 4) + 97],
                        xwin(xt, C + 1, par, 4 * m, 4),
                        wv_ext,
                        start=True, stop=True)
                eng = bal.pick(776)
                pvs = pv[:, 8 * par:8 * par + 8, :]
                _copy(nc, eng,
                      bass.AP(tensor=pvs.tensor, offset=pvs.offset,
                              ap=[list(pvs.ap[0]), [388, 2], [1, 388]]),
                      bass.AP(tensor=v_ps.tensor, offset=v_ps.offset,
                              ap=[list(v_ps.ap[0]), [512, 2], [1, 388]]))
            return pv

        def attention_parity(b, par, g, pv, fill):
            """S -> exp -> mask -> AV -> drain -> DMA for one parity.
            `fill`: list of emit-callbacks for next image's qk2/v work,
            interleaved here to keep PE busy during exp/mask latency."""
            xt = xtp[b % 2]
            sps = pss.tile([128, 2, 512], F32, tag="s")
            for k in range(NB):
                cols = slice(56 * (k % 7), 56 * (k % 7) + 56)
                rhs = g[:, par, 2 * k:2 * k + 2, :]
                if k % 2 == 0:
                    nc.tensor.matmul(sps[:, k // 7, cols],
                                     xwin(xt, C, par, 2 * k, 4),
                                     rhs, start=True, stop=True)
                else:
                    # swapped halves so AV operand partitions align with pv
                    nc.tensor.matmul(sps[64:128, k // 7, cols],
                                     xwin(xt, C, par, 2 * k, 2),
                                     rhs, start=True, stop=True)
                    nc.tensor.matmul(sps[0:64, k // 7, cols],
                                     xwin(xt, C, par, 2 * k + 2, 2),
                                     rhs, start=True, stop=True)
            for f in fill:
                f()
            e_t = epool.tile([128, 2, 7, 56], BF16, tag="E")
            nc.scalar.activation(
                bass.AP(tensor=e_t.tensor, offset=e_t.offset,
                        ap=[list(e_t.ap[0]), [392, 2], [1, 392]]),
                bass.AP(tensor=sps.tensor, offset=sps.offset,
                        ap=[list(sps.ap[0]), [512, 2], [1, 392]]),
                mybir.ActivationFunctionType.Exp, scale=SCALE)
            bal.charge(_ACT, 784 * 0.8333 + 185)
            em = epool.tile([128, 2, 7, 56], BF16, tag="EM")
            if par == 3:
                nc.vector.tensor_mul(
                    em.rearrange("p a b c -> p (a b) c"),
                    e_t.rearrange("p a b c -> p (a b) c"),
                    m_sb[:])
                bal.charge(_DVE, 784 * 0.52 + 60)
            else:
                nc.gpsimd.tensor_mul(
                    em.rearrange("p a b c -> p (a b) c"),
                    e_t.rearrange("p a b c -> p (a b) c"),
                    m_sb[:])

            ops = pso.tile([C + 1, 3, 512], F32, tag="av")
            for k in range(NB):
                j = k // 2
                if k % 2 == 0:
                    nc.tensor.matmul(ops[:, 0, 56 * j:56 * j + 56],
                                     pv[:, 8 * par + j, :],
                                     em[:, k // 7, k % 7, :],
                                     start=True, stop=True)
                else:
                    # window straddles pv groups j and j+1: two partials,
                    # summed during the drain (PE can't accumulate across
                    # operand base partitions; DVE reads one PSUM input max)
                    nc.tensor.matmul(ops[:, 1, 56 * j:56 * j + 56],
                                     pv[64:128, 8 * par + j, :],
                                     em[64:128, k // 7, k % 7, :],
                                     start=True, stop=True)
                    nc.tensor.matmul(ops[:, 2, 56 * j:56 * j + 56],
                                     pv[0:64, 8 * par + j + 1, :],
                                     em[0:64, k // 7, k % 7, :],
                                     start=True, stop=True)
            osb = opool.tile([C + 1, NB, 56], BF16, tag="osb")
            pitch = list(osb.ap[0])
            ev_dst = bass.AP(tensor=osb.tensor, offset=osb.offset,
                             ap=[pitch, [112, 7], [1, 56]])
            od_dst = bass.AP(tensor=osb.tensor, offset=osb.offset + 56,
                             ap=[pitch, [112, 7], [1, 56]])
            def bank(i):
                return bass.AP(tensor=ops.tensor, offset=ops.offset + 512 * i,
                               ap=[list(ops.ap[0]), [56, 7], [1, 56]])
            _copy(nc, bal.pick(392), ev_dst, bank(0))
            _copy(nc, bal.pick(392), od_dst, bank(1))
            nc.vector.tensor_add(od_dst, od_dst, bank(2))
            bal.charge(_DVE, 392 * 1.0417 + 125)
            nc.sync.dma_start(o_d[b, par], osb.rearrange("p a b -> p (a b)"))

        # software pipeline over images
        load_image(0)
        g = qk2_pass(0)
        pv = v_pass(0)
        for b in range(BPC):
            nxt = []
            if b + 1 < BPC:
                state = {}

                def mk_load(bb=b + 1):
                    return lambda: load_image(bb)

                def mk_qk2(bb=b + 1):
                    def f():
                        state["g"] = qk2_pass(bb)
                    return f

                def mk_v(bb=b + 1):
                    def f():
                        state["pv"] = v_pass(bb)
                    return f

                nxt = [mk_load(), mk_qk2(), mk_v(), lambda: None]
            else:
                nxt = [lambda: None] * 4
                state = {}
            for par in range(P):
                attention_parity(b, par, g, pv, [nxt[par]])
            if b + 1 < BPC:
                g = state["g"]
                pv = state["pv"]


def _host_consts():
    # band mask [128, 56]: pos (k in 0..3, w in 0..31); token (j in 0..1,
    # wt in 0..27); valid iff k-j in {0,1,2} and w-wt in {0,1,2}
    k = np.arange(4)[:, None, None, None]
    w = np.arange(PR)[None, :, None, None]
    j = np.arange(2)[None, None, :, None]
    wt = np.arange(R)[None, None, None, :]
    m = ((k - j >= 0) & (k - j <= 2) & (w - wt >= 0) & (w - wt <= 2))
    m_even = m.astype(np.float32).reshape(4 * PR, 56)
    # odd blocks: physical partition row kk holds logical window row (kk+2)%4
    m_odd = m_even.reshape(4, PR, 56)[[2, 3, 0, 1]].reshape(4 * PR, 56)
    out = np.zeros((4 * PR, NB, 56), dtype=np.float32)
    for kb in range(NB):
        out[:, kb, :] = m_even if kb % 2 == 0 else m_odd
    return out.astype(ml_dtypes.bfloat16)


def _host_pack_x(x):
    """x (B, N, C) f32 -> (B, 97, 4, 32, 30) bf16 padded parity layout."""
    xr = x.reshape(B, H, H, C)
    out = np.zeros((B, C + 1, P, RP, PR), dtype=np.float32)
    for a in range(2):
        for c in range(2):
            par = 2 * a + c
            sub = xr[:, a::2, c::2, :]            # (B, 28, 28, C)
            out[:, 0:C, par, 1:29, 1:29] = sub.transpose(0, 3, 1, 2)
    out[:, C, :, :, :] = 1.0
    return out.astype(ml_dtypes.bfloat16)


def _host_pack_w(W_qkv):
    wq = W_qkv[0:C, :]
    wk = W_qkv[C:2 * C, :]
    wv = W_qkv[2 * C:3 * C, :]
    wt = np.zeros((C + 1, 2 * C + 1), dtype=np.float32)
    wt[0:C, 0:C] = wq.T @ wk                  # wqkL: g = wqkL^T x
    wt[0:C, C:2 * C] = wv.T                   # v = wv_ext^T x_ext
    wt[C, 2 * C] = 1.0                        # den ones channel
    return wt.astype(ml_dtypes.bfloat16)


def _host_unpack_o(o):
    """o (ncores, bpc, P, 97, 784) -> (ncores*bpc, N, C) f32, num/den divide."""
    o = np.asarray(o, dtype=np.float32)
    nc_, bpc = o.shape[0], o.shape[1]
    num = o[:, :, :, 0:C, :]
    den = o[:, :, :, C:C + 1, :]
    res = num / den                            # (nc, bpc, 4, 96, 784)
    res = res.reshape(nc_, bpc, 2, 2, C, R, R)
    y = np.zeros((nc_, bpc, H, H, C), dtype=np.float32)
    for a in range(2):
        for c in range(2):
            y[:, :, a::2, c::2, :] = res[:, :, a, c].transpose(0, 1, 3, 4, 2)
    return y.reshape(nc_ * bpc, N, C)


def kernel(x, W_qkv):
    x = np.asarray(x, dtype=np.float32)
    W_qkv = np.asarray(W_qkv, dtype=np.float32)

    if "nc" not in _NC_CACHE:
        _NC_CACHE["nc"] = build_nc()
    nc = _NC_CACHE["nc"]

    xt = _host_pack_x(x).reshape(NCORES, BPC, C + 1, P, RP, PR)
    wt = _host_pack_w(W_qkv)
    mk = _host_consts()

    in_maps = [{"xt": xt[i], "wt": wt, "mask": mk} for i in range(NCORES)]
    bkr = run_bass_kernel_spmd(nc, in_maps, list(range(NCORES)))
    _NC_CACHE["last_results"] = bkr
    o = np.stack([np.asarray(r["o"]) for r in bkr.results])
    return np.ascontiguousarray(_host_unpack_o(o).astype(np.float32))

